# revision 38
# baseline (speedup 1.0000x reference)
"""Causal multi-head attention (B=2, H=16, S=2048, D=128, fp32) on 8 NeuronCores.

Sharding: the 32 (batch, head) pairs are split 4-per-core (tensor parallel over
heads, data parallel over batch — both collapse to the fused pair axis).

Per-core kernel (per pair), flash-attention style without max-subtraction
(scores have unit variance, so exp never overflows in fp32):

  scores_T[k, q] = K_blk^T.T @ Q^T            (bf16 matmuls into fp32 PSUM,
                                               causally trimmed free dim;
                                               Q^T prescaled by SCALE/16 so
                                               PSUM holds u = s/16)
  P_T = exp(s)                                SPLIT between ScalarE (Exp
                                              activation, scale=16) and DVE
                                              (custom EXP16_POLY_ANT op:
                                              deg-2 poly in u then 4
                                              squarings = exp(16u)·(1+2%)),
                                              strips routed by greedy balance;
                                              DVE only gets strips whose
                                              min row has >=512 keys so the
                                              softmax normalization washes
                                              out the poly ripple.
  causal mask on diagonal 128x128 blocks      (a -8 strict-upper-triangular
                                               bias is matmul-accumulated into
                                               the diagonal score block — tri
                                               stationary x identity moving —
                                               so ScalarE exp(16u-128) gives
                                               exact zeros; no mask multiply)
  ctx[q, 0:128] , l[q] = P_T_blk.T @ [V | 1]  (bf16 matmuls, PSUM-accumulated
                                               over k blocks; the ones column
                                               of V_aug yields the softmax
                                               denominator for free)
  out[q, :] = ctx[q, :] / l[q]                (DVE batched reciprocal per PSUM
                                               bank + per-group scalar multiply
                                               assigned to ScalarE or DVE by
                                               running balance)

Scheduling notes: scores for block kb+1 are emitted before PV matmuls of block
kb so the PE FIFO keeps the exp engines fed; alternating strip routing keeps
ScalarE and DVE concurrently busy. PSUM start=True clears has_written for a
whole bank, so of the 8 packed ctx accumulation groups only the first per bank
(s=0/3/6) uses start=True and the rest rely on overwrite-on-first-touch.

Q^T / K^T (bf16) and the bf16 [V | 1] augmentation are prepared host-side in
kernel() — host preprocessing is part of the sharding step.
"""

import math

import ml_dtypes
import numpy as np

import concourse.bass as bass
import concourse.mybir as mybir
from concourse import bacc, tile
from concourse.bass_utils import run_bass_kernel_spmd

# ---- custom DVE op: EXP16_POLY_ANT -----------------------------------------
# out = ((C0*u + C1)*u + C2)^16 ~= exp(16u) for u = s/16, accurate to ~2% for
# |s| <= 4.5 with graceful degradation in the tails (poly is positive
# everywhere: discriminant < 0). Registered into concourse.dve_ops at import.
from concourse.dve_spec import Spec, Src0, C0, C1, C2, lower, sq
from concourse.dve_uop import DveOpSpec
import concourse.dve_ops as dve_ops_mod
from concourse.dve_ops import DveOp

_u = Src0
_body = sq(sq(sq(sq((_u * C0 + C1) * _u + C2))))


def _exp16_ref(in0, in1, s0, s1, imm2):
    u = in0.astype(np.float32)
    p = ((u * np.float32(s0) + np.float32(s1)) * u + np.float32(imm2)).astype(
        np.float32
    )
    for _ in range(4):
        p = (p * p).astype(np.float32)
    return p


_spec = Spec(body=_body, reference=_exp16_ref)
_EXP16_NAME = "EXP16_POLY_ANT"
_EXP16_ROW = 17  # rows [1, 0x20) free; production ops use 1..16
_sha = DveOpSpec(
    name=_EXP16_NAME, opcode=_EXP16_ROW, uops=lower(_spec, ver="v3"), rd1_en=False
).sha("v3")
EXP16 = DveOp(_EXP16_NAME, _spec, subdim=False, uops_sha={"v3": _sha})
if _EXP16_NAME not in dve_ops_mod._SUB_OPCODE_FOR_NAME:
    dve_ops_mod.OPS.append(EXP16)
    dve_ops_mod.CUSTOM_DVE_SPECS[_EXP16_NAME] = _spec
    dve_ops_mod._SUB_OPCODE_FOR_NAME[_EXP16_NAME] = _EXP16_ROW

# minimax-relative fit of exp(u) on |u| <= 4.5/16 (see transcript)
B2, B1, B0 = 0.49894845, 1.0117635, 1.000231
# shifted variant for the fp8 chunks: p2(u - G)^16 ~= exp(16u - GAMMA), which
# keeps exp outputs within fp8e4m3 range (max ~e^6 < 448); the common e^-GAMMA
# factor cancels in the softmax normalization
GAMMA = 1.0
_G = GAMMA / 16.0
C0d = B2
C1d = B1 - 2.0 * _G * B2
C2d = B2 * _G * _G - B1 * _G + B0

B, H, S, D = 2, 16, 2048, 128
NCORES = 8
NPAIRS = B * H              # 32 fused (batch, head) pairs
PPC = NPAIRS // NCORES      # 4 pairs per core
KB = 128                    # k block (PE contraction / partition dim)
QC = 1024                   # q chunk (scores psum free dim)
NSUB = QC // 128            # sub-q blocks (PV stationary width) per chunk
NKT = S // KB               # 16 k blocks per sequence
SCALE = 1.0 / math.sqrt(D)  # net score scale: /(sqrt(d)*coeff) then *coeff
PRE = SCALE / 16.0          # host-side Q prescale; PSUM holds u = s/16

F32 = mybir.dt.float32
BF16 = mybir.dt.bfloat16
FP8 = mybir.dt.float8e4

# modeled engine-op costs (ns) for the static balancer
_NS_COL_S = 0.833   # ScalarE activation per free column
_NS_COL_D = 1.042   # DVE per free column
_OV_S = 185.0       # ScalarE per-instruction overhead
_OV_D = 130.0       # DVE per-instruction overhead
_NORM_S = 380.0     # ScalarE Copy+scale [128,128] (measured)
_NORM_D = 322.0     # DVE tensor_scalar_mul [128,128] (measured)


def _build_nc():
    nc = bacc.Bacc("TRN2", target_bir_lowering=False, debug=False)
    qt_d = nc.dram_tensor("qt", [PPC, D, S], BF16, kind="ExternalInput")
    kt_d = nc.dram_tensor("kt", [PPC, D, S], BF16, kind="ExternalInput")
    va_d = nc.dram_tensor("va", [PPC, KB, NKT // 2, KB + 1], BF16, kind="ExternalInput")
    va2_d = nc.dram_tensor(
        "va2", [PPC, KB, NKT // 2, 2, KB + 1], FP8, kind="ExternalInput"
    )
    # partition-major output layout [pair, q%128, q//128, d]: each SBUF
    # partition contributes one contiguous (groups x 512B) run per store, so
    # store descriptors are nsb*512B instead of 512B; the host transposes back
    out_d = nc.dram_tensor("out", [PPC, KB, NKT, D], BF16, kind="ExternalOutput")

    # Raw-bass warmup activation before the Tile body: places the ~1.3us ACT
    # table load in the preamble, off the first chunk's critical path.
    warm_sb = nc.alloc_sbuf_tensor("warm_sb", [128, 1], F32)
    nc.scalar.activation(
        warm_sb.ap(), warm_sb.ap(), mybir.ActivationFunctionType.Exp, scale=0.0
    )

    # running engine-busy estimates for greedy strip/normalize routing
    busy = {"S": 0.0, "D": 0.0}

    with tile.TileContext(nc) as tc:
        with (
            tc.tile_pool(name="cm", bufs=1) as c_pool,
            tc.tile_pool(name="qk", bufs=4) as qk_pool,
            tc.tile_pool(name="vp", bufs=3) as v_pool,
            tc.tile_pool(name="pp", bufs=12) as p_pool,
            tc.tile_pool(name="oo", bufs=8) as o_pool,
            tc.tile_pool(name="rr", bufs=8) as r_pool,
            tc.tile_pool(name="ps_s", bufs=5, space="PSUM") as ps_s,
            tc.tile_pool(name="ps_c", bufs=1, space="PSUM") as ps_c,
            tc.tile_pool(name="ps_c2", bufs=1, space="PSUM") as ps_c2,
        ):
            # constant operands for the diagonal-block causal bias matmul:
            # tri[p, m] = -8 iff m > p (else 0); ident[p, m] = (p == m).
            # matmul(sc_diag, tri, ident) accumulates -8 (= -128 in score
            # units at the 1/16 prescale) onto masked entries.
            tri_t = c_pool.tile([KB, KB], BF16, name="tri_t")
            nc.gpsimd.memset(tri_t[:], -8.0)
            nc.gpsimd.affine_select(
                out=tri_t[:],
                in_=tri_t[:],
                compare_op=mybir.AluOpType.is_gt,
                fill=0.0,
                base=0,
                pattern=[[1, KB]],
                channel_multiplier=-1,
            )
            ident_t = c_pool.tile([KB, KB], BF16, name="ident_t")
            nc.gpsimd.memset(ident_t[:], 1.0)
            nc.gpsimd.affine_select(
                out=ident_t[:],
                in_=ident_t[:],
                compare_op=mybir.AluOpType.is_equal,
                fill=0.0,
                base=0,
                pattern=[[1, KB]],
                channel_multiplier=-1,
            )
            # per-partition bias constant for the fp8 chunks' ScalarE exp
            nb_t = c_pool.tile([KB, 1], F32, name="nb_t")
            nc.gpsimd.memset(nb_t[:], -GAMMA)
            for p in range(PPC):
                qt_t = qk_pool.tile([D, S], BF16, tag="qt")
                kt_t = qk_pool.tile([D, S], BF16, tag="kt")
                va_t = v_pool.tile([KB, NKT // 2, KB + 1], BF16, tag="va")
                va2_t = v_pool.tile([KB, NKT // 2, 2, KB + 1], FP8, tag="va2")
                # asymmetric piecewise loads (region deps): a small leading
                # piece unblocks the first score matmuls early, the rest
                # follows in one transfer per tensor
                nc.sync.dma_start(out=kt_t[:, 0:256], in_=kt_d[p][:, 0:256])
                nc.sync.dma_start(out=qt_t[:, 0:1024], in_=qt_d[p][:, 0:1024])
                nc.sync.dma_start(out=va_t[:, 0:2, :], in_=va_d[p][:, 0:2, :])
                nc.sync.dma_start(out=kt_t[:, 256:], in_=kt_d[p][:, 256:])
                nc.sync.dma_start(out=qt_t[:, 1024:], in_=qt_d[p][:, 1024:])
                nc.sync.dma_start(out=va_t[:, 2:, :], in_=va_d[p][:, 2:, :])
                nc.sync.dma_start(out=va2_t[:], in_=va2_d[p])

                # last pair: big chunk first so the kernel tail is the small
                # chunk's short PV backlog
                qc_order = range(S // QC) if p < PPC - 1 else reversed(range(S // QC))
                for qc in qc_order:
                    q0 = qc * QC
                    # 8 ctx accumulators [128q, D+1], packed 3/3/2 per PSUM
                    # bank; ctx2 (stops last) double-buffered
                    ctx_tiles = [
                        ps_c.tile([128, 512], F32, tag="ctx0", name="ctx0"),
                        ps_c.tile([128, 512], F32, tag="ctx1", name="ctx1"),
                        ps_c2.tile([128, 512], F32, tag="ctx2", name="ctx2"),
                    ]

                    def ctx_ap(s):
                        t, i = divmod(s, 3)
                        return ctx_tiles[t][:, i * (KB + 1):(i + 1) * (KB + 1)]

                    nkb = (q0 + QC) // KB

                    # half-strip stream: (kb, hh) with live columns
                    # [c0, c1) of the strip; one PSUM bank per half so the
                    # score pipeline can run LOOKAHEAD halves deep,
                    # decoupling the PE round-trip latency from the exp
                    # engines.
                    halves = []
                    for kb in range(nkb):
                        lo = max(kb * KB - q0, 0)
                        for hh in range(QC // 512):
                            c0 = max(hh * 512, lo)
                            c1 = (hh + 1) * 512
                            if c0 < c1:
                                halves.append((kb, hh, c0, c1))

                    pt_tiles = {}
                    sc_tiles = {}

                    def emit_half_scores(i):
                        kb, hh, c0, c1 = halves[i]
                        k0 = kb * KB
                        off = k0 - q0
                        sc = ps_s.tile([KB, 512], F32, tag="sc", name="sc")
                        nc.tensor.matmul(
                            sc[:, c0 - hh * 512:c1 - hh * 512],
                            kt_t[:, k0:k0 + KB],
                            qt_t[:, q0 + c0:q0 + c1],
                            start=True,
                            stop=True,
                        )
                        if hh * 512 <= off < c1:
                            # causal bias on the diagonal 128x128 block
                            b0 = off - hh * 512
                            nc.tensor.matmul(
                                sc[:, b0:b0 + KB],
                                tri_t[:],
                                ident_t[:],
                                start=False,
                                stop=True,
                                skip_group_check=True,
                            )
                        sc_tiles[i] = sc

                    LOOKAHEAD = 4
                    for i in range(min(LOOKAHEAD, len(halves))):
                        emit_half_scores(i)
                    fp8_chunk = q0 > 0  # deep rows: fp8 P/V + DoubleRow PV

                    for i, (kb, hh, c0, c1) in enumerate(halves):
                        off = kb * KB - q0  # >= 0 on diagonal strips
                        sc = sc_tiles.pop(i)
                        if not fp8_chunk:
                            if kb not in pt_tiles:
                                pt_tiles[kb] = p_pool.tile(
                                    [KB, QC], BF16, tag="pt", name="pt"
                                )
                            dst = pt_tiles[kb][:, c0:c1]
                        else:
                            t = kb // 2
                            if t not in pt_tiles:
                                pt2 = p_pool.tile(
                                    [KB, 2, QC], FP8, tag="pt2", name="pt2"
                                )
                                pt_tiles[t] = pt2
                                lo_odd = max((2 * t + 1) * KB - q0, 0)
                                if lo_odd >= KB:
                                    # the odd strip's k-tile is dead below its
                                    # causal start; its paired group's
                                    # stationary slice must read zeros
                                    nc.gpsimd.memset(
                                        pt2[:, 1, lo_odd - KB:lo_odd], 0.0
                                    )
                            dst = pt_tiles[t][:, kb % 2, c0:c1]
                        lo_h, w = c0 - hh * 512, c1 - c0
                        has_diag = hh * 512 <= off < c1
                        # routing: the bias-masked diagonal half needs
                        # ScalarE's true exp; off-diagonal halves with >= 512
                        # keys for every row go to whichever engine is less
                        # busy (greedy min-makespan)
                        eligible = (not has_diag) and (q0 + c0) >= 512
                        cost_s = w * _NS_COL_S + _OV_S
                        cost_d = w * _NS_COL_D + _OV_D
                        if eligible and busy["D"] + cost_d <= busy["S"] + cost_s:
                            busy["D"] += cost_d
                            nc.vector._custom_dve(
                                EXP16,
                                out=dst,
                                in0=sc[:, lo_h:lo_h + w],
                                s0=float(C0d if fp8_chunk else B2),
                                s1=float(C1d if fp8_chunk else B1),
                                imm2=float(C2d if fp8_chunk else B0),
                            )
                        else:
                            busy["S"] += cost_s
                            nc.scalar.activation(
                                dst,
                                sc[:, lo_h:lo_h + w],
                                mybir.ActivationFunctionType.Exp,
                                scale=16.0,
                                bias=nb_t[:] if fp8_chunk else 0.0,
                            )
                        if i + LOOKAHEAD < len(halves):
                            emit_half_scores(i + LOOKAHEAD)
                        # PV matmuls for the sub-q blocks of this half
                        if fp8_chunk and kb % 2 == 0:
                            continue  # fp8 PVs fire on the pair's odd strip
                        if fp8_chunk:
                            t = kb // 2
                            pv_lo = max((kb - 1) * KB - q0, 0)
                        else:
                            pv_lo = c0
                        for s in range(hh * 4, hh * 4 + 4):
                            qs0 = s * 128
                            if qs0 < pv_lo:
                                continue  # sub-q fully masked for this k block
                            if fp8_chunk:
                                stop_t = (q0 // KB + s) // 2
                                nc.tensor.matmul(
                                    ctx_ap(s),
                                    pt_tiles[t][:, :, qs0:qs0 + 128],
                                    va2_t[:, t, :, :],
                                    start=(t == 0 and s % 3 == 0),
                                    stop=(t == stop_t),
                                    perf_mode=mybir.MatmulPerfMode.DoubleRow,
                                    skip_group_check=True,
                                )
                                is_stop = t == stop_t
                            else:
                                last_kb = q0 // KB + s
                                nc.tensor.matmul(
                                    ctx_ap(s),
                                    pt_tiles[kb][:, qs0:qs0 + 128],
                                    va_t[:, kb, :],
                                    start=(kb == 0 and s % 3 == 0),
                                    stop=(kb == last_kb),
                                    skip_group_check=True,
                                )
                                is_stop = kb == last_kb
                            # normalize + store a ctx bank as soon as its
                            # last accumulation group stopped
                            for bank, s_hi in ((0, 2), (1, 5), (2, 7)):
                                if s != s_hi or not is_stop:
                                    continue
                                s_lo = 3 * bank
                                nsb = s_hi - s_lo + 1
                                # drain the bank PSUM->SBUF in one batched
                                # copy (ScalarE or DVE, whichever is lighter),
                                # reciprocal from SBUF on DVE, then the idle
                                # GpSimd applies 1/l with a free-dim-broadcast
                                # tensor multiply (GpSimd cannot read PSUM,
                                # and its tensor_scalar path is slow, but
                                # plain tensor_tensor runs at ~0.42 eff)
                                raw = o_pool.tile(
                                    [128, 3, KB + 1], F32, tag="raw", name="raw"
                                )
                                ob = o_pool.tile([128, 3, D], BF16, tag="ob")
                                rec = r_pool.tile([128, 3], F32, tag="rec")
                                cb = ctx_tiles[bank][
                                    :, 0:nsb * (KB + 1)
                                ].rearrange("p (g c) -> p g c", c=KB + 1)
                                cost_s = nsb * (KB + 1) * _NS_COL_S + _OV_S
                                cost_d = nsb * (KB + 1) * _NS_COL_D + _OV_D
                                if busy["S"] + cost_s <= busy["D"] + cost_d:
                                    busy["S"] += cost_s
                                    nc.scalar.copy(raw[:, 0:nsb, :], cb)
                                else:
                                    busy["D"] += cost_d
                                    nc.vector.tensor_copy(raw[:, 0:nsb, :], cb)
                                busy["D"] += _OV_D
                                nc.vector.reciprocal(
                                    rec[:, 0:nsb], raw[:, 0:nsb, KB]
                                )
                                for j in range(nsb):
                                    nc.gpsimd.tensor_mul(
                                        ob[:, j, :],
                                        raw[:, j, 0:D],
                                        rec[:, j:j + 1].to_broadcast([128, D]),
                                    )
                                g_lo = q0 // KB + s_lo
                                nc.sync.dma_start(
                                    out=out_d[p, :, g_lo:g_lo + nsb, :],
                                    in_=ob[:, 0:nsb, :],
                                )
    nc.compile()
    return nc


def _prep_inputs(query_layer, key_layer, value_layer):
    q = np.asarray(query_layer, dtype=np.float32).reshape(NPAIRS, S, D)
    k = np.asarray(key_layer, dtype=np.float32).reshape(NPAIRS, S, D)
    v = np.asarray(value_layer, dtype=np.float32).reshape(NPAIRS, S, D)

    qt = np.ascontiguousarray((q * np.float32(PRE)).transpose(0, 2, 1)).astype(
        ml_dtypes.bfloat16
    )
    kt = np.ascontiguousarray(k.transpose(0, 2, 1)).astype(ml_dtypes.bfloat16)
    vk = v.reshape(NPAIRS, NKT, KB, D).transpose(0, 2, 1, 3)  # [N, k, kb, D]
    # chunk-0 (rows < 1024) PV operand: bf16 [V | 1], k blocks 0..7
    va = np.ones((NPAIRS, KB, NKT // 2, KB + 1), dtype=ml_dtypes.bfloat16)
    va[:, :, :, :D] = vk[:, :, : NKT // 2].astype(ml_dtypes.bfloat16)
    # chunk-1 (rows >= 1024) PV operand: fp8 [V | 1], all k blocks packed in
    # pairs for DoubleRow (contraction 2x128)
    fp8 = ml_dtypes.float8_e4m3fn
    va2 = np.ones((NPAIRS, KB, NKT // 2, 2, KB + 1), dtype=fp8)
    va2[:, :, :, :, :D] = (
        vk.reshape(NPAIRS, KB, NKT // 2, 2, D).astype(fp8)
    )
    in_maps = [
        {
            "qt": np.ascontiguousarray(qt[c * PPC:(c + 1) * PPC]),
            "kt": np.ascontiguousarray(kt[c * PPC:(c + 1) * PPC]),
            "va": np.ascontiguousarray(va[c * PPC:(c + 1) * PPC]),
            "va2": np.ascontiguousarray(va2[c * PPC:(c + 1) * PPC]),
        }
        for c in range(NCORES)
    ]
    return in_maps


def _run(query_layer, key_layer, value_layer, trace=False):
    in_maps = _prep_inputs(query_layer, key_layer, value_layer)
    nc = _build_nc()
    res = run_bass_kernel_spmd(nc, in_maps, list(range(NCORES)), trace=trace)
    # device layout [core, pair, q%128, q//128, d] -> [b, s, h*d]
    ctx = np.stack([res.results[c]["out"] for c in range(NCORES)])
    ctx = ctx.transpose(0, 1, 3, 2, 4)  # [core, pair, q//128, q%128, d]
    out = ctx.reshape(B, H, S, D).transpose(0, 2, 1, 3).reshape(B, S, H * D)
    return np.ascontiguousarray(out, dtype=np.float32), res


def kernel(query_layer, key_layer, value_layer):
    out, _ = _run(query_layer, key_layer, value_layer, trace=False)
    return out


# revision 39
# speedup vs baseline: 1.0094x; 1.0094x over previous
"""Causal multi-head attention (B=2, H=16, S=2048, D=128, fp32) on 8 NeuronCores.

Sharding: the 32 (batch, head) pairs are split 4-per-core (tensor parallel over
heads, data parallel over batch — both collapse to the fused pair axis).

Per-core kernel (per pair), flash-attention style without max-subtraction
(scores have unit variance, so exp never overflows in fp32):

  scores_T[k, q] = K_blk^T.T @ Q^T            (bf16 matmuls into fp32 PSUM,
                                               causally trimmed free dim;
                                               Q^T prescaled by SCALE/16 so
                                               PSUM holds u = s/16)
  P_T = exp(s)                                SPLIT between ScalarE (Exp
                                              activation, scale=16) and DVE
                                              (custom EXP16_POLY_ANT op:
                                              deg-2 poly in u then 4
                                              squarings = exp(16u)·(1+2%)),
                                              strips routed by greedy balance;
                                              DVE only gets strips whose
                                              min row has >=512 keys so the
                                              softmax normalization washes
                                              out the poly ripple.
  causal mask on diagonal 128x128 blocks      (a -8 strict-upper-triangular
                                               bias is matmul-accumulated into
                                               the diagonal score block — tri
                                               stationary x identity moving —
                                               so ScalarE exp(16u-128) gives
                                               exact zeros; no mask multiply)
  ctx[q, 0:128] , l[q] = P_T_blk.T @ [V | 1]  (PSUM-accumulated over k blocks;
                                               the ones column of V_aug yields
                                               the softmax denominator free.
                                               Chunk 0 (rows < 1024): bf16,
                                               one 129-wide matmul per k
                                               block. Chunk 1 (rows >= 1024):
                                               fp8e4m3 P and V with exp(s-1)
                                               scaling — range fits e4m3, the
                                               e^-1 cancels in normalize —
                                               and DoubleRow matmuls packing
                                               two k blocks per pass
                                               (contraction 2x128) at 2x PE
                                               throughput)
  out[q, :] = ctx[q, :] / l[q]                (ScalarE/DVE batched bank drain
                                               PSUM->SBUF, DVE reciprocal,
                                               GpSimd broadcast tensor-mul,
                                               bf16 store in a partition-major
                                               DRAM layout for 4x fewer DMA
                                               descriptors; host transposes)

Scheduling notes: score strips are split into 512-col halves (one PSUM bank
each) so the score pipeline runs LOOKAHEAD=4 halves deep — decoupling the
PE->exp->PE round-trip latency from engine throughput (this was worth ~25us).
Scores for upcoming halves are emitted before the current half's PV matmuls so
the PE FIFO keeps the exp engines fed; greedy min-makespan routing keeps
ScalarE and DVE concurrently busy. PSUM start=True clears has_written for a
whole bank, so of the 8 packed ctx accumulation groups only the first per bank
(s=0/3/6) uses start=True and the rest rely on overwrite-on-first-touch. For
fp8 pairs whose odd strip starts mid-block, the dead 128 columns of the packed
stationary are memset to zero.

Q^T / K^T (bf16) and the [V | 1] augmentations (bf16 + packed fp8) are
prepared host-side in kernel() — host preprocessing is part of the sharding
step.
"""

import math

import ml_dtypes
import numpy as np

import concourse.bass as bass
import concourse.mybir as mybir
from concourse import bacc, tile
from concourse.bass_utils import run_bass_kernel_spmd

# ---- custom DVE op: EXP16_POLY_ANT -----------------------------------------
# out = ((C0*u + C1)*u + C2)^16 ~= exp(16u) for u = s/16, accurate to ~2% for
# |s| <= 4.5 with graceful degradation in the tails (poly is positive
# everywhere: discriminant < 0). Registered into concourse.dve_ops at import.
from concourse.dve_spec import Spec, Src0, C0, C1, C2, lower, sq
from concourse.dve_uop import DveOpSpec
import concourse.dve_ops as dve_ops_mod
from concourse.dve_ops import DveOp

_u = Src0
_body = sq(sq(sq(sq((_u * C0 + C1) * _u + C2))))


def _exp16_ref(in0, in1, s0, s1, imm2):
    u = in0.astype(np.float32)
    p = ((u * np.float32(s0) + np.float32(s1)) * u + np.float32(imm2)).astype(
        np.float32
    )
    for _ in range(4):
        p = (p * p).astype(np.float32)
    return p


_spec = Spec(body=_body, reference=_exp16_ref)
_EXP16_NAME = "EXP16_POLY_ANT"
_EXP16_ROW = 17  # rows [1, 0x20) free; production ops use 1..16
_sha = DveOpSpec(
    name=_EXP16_NAME, opcode=_EXP16_ROW, uops=lower(_spec, ver="v3"), rd1_en=False
).sha("v3")
EXP16 = DveOp(_EXP16_NAME, _spec, subdim=False, uops_sha={"v3": _sha})
if _EXP16_NAME not in dve_ops_mod._SUB_OPCODE_FOR_NAME:
    dve_ops_mod.OPS.append(EXP16)
    dve_ops_mod.CUSTOM_DVE_SPECS[_EXP16_NAME] = _spec
    dve_ops_mod._SUB_OPCODE_FOR_NAME[_EXP16_NAME] = _EXP16_ROW

# minimax-relative fit of exp(u) on |u| <= 4.5/16 (see transcript)
B2, B1, B0 = 0.49894845, 1.0117635, 1.000231
# shifted variant for the fp8 chunks: p2(u - G)^16 ~= exp(16u - GAMMA), which
# keeps exp outputs within fp8e4m3 range (max ~e^6 < 448); the common e^-GAMMA
# factor cancels in the softmax normalization
GAMMA = 1.0
_G = GAMMA / 16.0
C0d = B2
C1d = B1 - 2.0 * _G * B2
C2d = B2 * _G * _G - B1 * _G + B0

B, H, S, D = 2, 16, 2048, 128
NCORES = 8
NPAIRS = B * H              # 32 fused (batch, head) pairs
PPC = NPAIRS // NCORES      # 4 pairs per core
KB = 128                    # k block (PE contraction / partition dim)
QC = 1024                   # q chunk (scores psum free dim)
NSUB = QC // 128            # sub-q blocks (PV stationary width) per chunk
NKT = S // KB               # 16 k blocks per sequence
SCALE = 1.0 / math.sqrt(D)  # net score scale: /(sqrt(d)*coeff) then *coeff
PRE = SCALE / 16.0          # host-side Q prescale; PSUM holds u = s/16

F32 = mybir.dt.float32
BF16 = mybir.dt.bfloat16
FP8 = mybir.dt.float8e4

# modeled engine-op costs (ns) for the static balancer
_NS_COL_S = 0.833   # ScalarE activation per free column
_NS_COL_D = 1.042   # DVE per free column
_OV_S = 185.0       # ScalarE per-instruction overhead
_OV_D = 130.0       # DVE per-instruction overhead
_NORM_S = 380.0     # ScalarE Copy+scale [128,128] (measured)
_NORM_D = 322.0     # DVE tensor_scalar_mul [128,128] (measured)


def _build_nc():
    nc = bacc.Bacc("TRN2", target_bir_lowering=False, debug=False)
    qt_d = nc.dram_tensor("qt", [PPC, D, S], BF16, kind="ExternalInput")
    kt_d = nc.dram_tensor("kt", [PPC, D, S], BF16, kind="ExternalInput")
    va_d = nc.dram_tensor("va", [PPC, KB, NKT // 2, KB + 1], BF16, kind="ExternalInput")
    va2_d = nc.dram_tensor(
        "va2", [PPC, KB, NKT // 2, 2, KB + 1], FP8, kind="ExternalInput"
    )
    # partition-major output layout [pair, q%128, q//128, d]: each SBUF
    # partition contributes one contiguous (groups x 512B) run per store, so
    # store descriptors are nsb*512B instead of 512B; the host transposes back
    out_d = nc.dram_tensor("out", [PPC, KB, NKT, D], BF16, kind="ExternalOutput")

    # Raw-bass warmup activation before the Tile body: places the ~1.3us ACT
    # table load in the preamble, off the first chunk's critical path.
    warm_sb = nc.alloc_sbuf_tensor("warm_sb", [128, 1], F32)
    nc.scalar.activation(
        warm_sb.ap(), warm_sb.ap(), mybir.ActivationFunctionType.Exp, scale=0.0
    )

    # running engine-busy estimates for greedy strip/normalize routing
    busy = {"S": 0.0, "D": 0.0}

    with tile.TileContext(nc) as tc:
        with (
            tc.tile_pool(name="cm", bufs=1) as c_pool,
            tc.tile_pool(name="qk", bufs=4) as qk_pool,
            tc.tile_pool(name="vp", bufs=3) as v_pool,
            tc.tile_pool(name="pp", bufs=12) as p_pool,
            tc.tile_pool(name="oo", bufs=8) as o_pool,
            tc.tile_pool(name="rr", bufs=8) as r_pool,
            tc.tile_pool(name="ps_s", bufs=5, space="PSUM") as ps_s,
            tc.tile_pool(name="ps_c", bufs=1, space="PSUM") as ps_c,
            tc.tile_pool(name="ps_c2", bufs=1, space="PSUM") as ps_c2,
        ):
            # constant operands for the diagonal-block causal bias matmul:
            # tri[p, m] = -8 iff m > p (else 0); ident[p, m] = (p == m).
            # matmul(sc_diag, tri, ident) accumulates -8 (= -128 in score
            # units at the 1/16 prescale) onto masked entries.
            tri_t = c_pool.tile([KB, KB], BF16, name="tri_t")
            nc.gpsimd.memset(tri_t[:], -8.0)
            nc.gpsimd.affine_select(
                out=tri_t[:],
                in_=tri_t[:],
                compare_op=mybir.AluOpType.is_gt,
                fill=0.0,
                base=0,
                pattern=[[1, KB]],
                channel_multiplier=-1,
            )
            ident_t = c_pool.tile([KB, KB], BF16, name="ident_t")
            nc.gpsimd.memset(ident_t[:], 1.0)
            nc.gpsimd.affine_select(
                out=ident_t[:],
                in_=ident_t[:],
                compare_op=mybir.AluOpType.is_equal,
                fill=0.0,
                base=0,
                pattern=[[1, KB]],
                channel_multiplier=-1,
            )
            # per-partition bias constant for the fp8 chunks' ScalarE exp
            nb_t = c_pool.tile([KB, 1], F32, name="nb_t")
            nc.gpsimd.memset(nb_t[:], -GAMMA)
            for p in range(PPC):
                qt_t = qk_pool.tile([D, S], BF16, tag="qt")
                kt_t = qk_pool.tile([D, S], BF16, tag="kt")
                va_t = v_pool.tile([KB, NKT // 2, KB + 1], BF16, tag="va")
                va2_t = v_pool.tile([KB, NKT // 2, 2, KB + 1], FP8, tag="va2")
                # asymmetric piecewise loads (region deps): a small leading
                # piece unblocks the first score matmuls early, the rest
                # follows in one transfer per tensor
                nc.sync.dma_start(out=kt_t[:, 0:256], in_=kt_d[p][:, 0:256])
                nc.sync.dma_start(out=qt_t[:, 0:1024], in_=qt_d[p][:, 0:1024])
                nc.sync.dma_start(out=va_t[:, 0:2, :], in_=va_d[p][:, 0:2, :])
                nc.sync.dma_start(out=kt_t[:, 256:], in_=kt_d[p][:, 256:])
                nc.sync.dma_start(out=qt_t[:, 1024:], in_=qt_d[p][:, 1024:])
                nc.sync.dma_start(out=va_t[:, 2:, :], in_=va_d[p][:, 2:, :])
                nc.sync.dma_start(out=va2_t[:], in_=va2_d[p])

                # last pair: big chunk first so the kernel tail is the small
                # chunk's short PV backlog
                qc_order = range(S // QC) if p < PPC - 1 else reversed(range(S // QC))
                for qc in qc_order:
                    q0 = qc * QC
                    # 8 ctx accumulators [128q, D+1], packed 3/3/2 per PSUM
                    # bank; ctx2 (stops last) double-buffered
                    ctx_tiles = [
                        ps_c.tile([128, 512], F32, tag="ctx0", name="ctx0"),
                        ps_c.tile([128, 512], F32, tag="ctx1", name="ctx1"),
                        ps_c2.tile([128, 512], F32, tag="ctx2", name="ctx2"),
                    ]

                    def ctx_ap(s):
                        t, i = divmod(s, 3)
                        return ctx_tiles[t][:, i * (KB + 1):(i + 1) * (KB + 1)]

                    nkb = (q0 + QC) // KB

                    # half-strip stream: (kb, hh) with live columns
                    # [c0, c1) of the strip; one PSUM bank per half so the
                    # score pipeline can run LOOKAHEAD halves deep,
                    # decoupling the PE round-trip latency from the exp
                    # engines.
                    halves = []
                    for kb in range(nkb):
                        lo = max(kb * KB - q0, 0)
                        for hh in range(QC // 512):
                            c0 = max(hh * 512, lo)
                            c1 = (hh + 1) * 512
                            if c0 < c1:
                                halves.append((kb, hh, c0, c1))

                    pt_tiles = {}
                    sc_tiles = {}

                    def emit_half_scores(i):
                        kb, hh, c0, c1 = halves[i]
                        k0 = kb * KB
                        off = k0 - q0
                        sc = ps_s.tile([KB, 512], F32, tag="sc", name="sc")
                        nc.tensor.matmul(
                            sc[:, c0 - hh * 512:c1 - hh * 512],
                            kt_t[:, k0:k0 + KB],
                            qt_t[:, q0 + c0:q0 + c1],
                            start=True,
                            stop=True,
                        )
                        if hh * 512 <= off < c1:
                            # causal bias on the diagonal 128x128 block
                            b0 = off - hh * 512
                            nc.tensor.matmul(
                                sc[:, b0:b0 + KB],
                                tri_t[:],
                                ident_t[:],
                                start=False,
                                stop=True,
                                skip_group_check=True,
                            )
                        sc_tiles[i] = sc

                    LOOKAHEAD = 4
                    for i in range(min(LOOKAHEAD, len(halves))):
                        emit_half_scores(i)
                    fp8_chunk = q0 > 0  # deep rows: fp8 P/V + DoubleRow PV

                    for i, (kb, hh, c0, c1) in enumerate(halves):
                        off = kb * KB - q0  # >= 0 on diagonal strips
                        sc = sc_tiles.pop(i)
                        if not fp8_chunk:
                            if kb not in pt_tiles:
                                pt_tiles[kb] = p_pool.tile(
                                    [KB, QC], BF16, tag="pt", name="pt"
                                )
                            dst = pt_tiles[kb][:, c0:c1]
                        else:
                            t = kb // 2
                            if t not in pt_tiles:
                                pt2 = p_pool.tile(
                                    [KB, 2, QC], FP8, tag="pt2", name="pt2"
                                )
                                pt_tiles[t] = pt2
                                lo_odd = max((2 * t + 1) * KB - q0, 0)
                                if lo_odd >= KB:
                                    # the odd strip's k-tile is dead below its
                                    # causal start; its paired group's
                                    # stationary slice must read zeros
                                    nc.gpsimd.memset(
                                        pt2[:, 1, lo_odd - KB:lo_odd], 0.0
                                    )
                            dst = pt_tiles[t][:, kb % 2, c0:c1]
                        lo_h, w = c0 - hh * 512, c1 - c0
                        has_diag = hh * 512 <= off < c1
                        # routing: the bias-masked diagonal half needs
                        # ScalarE's true exp; off-diagonal halves with >= 512
                        # keys for every row go to whichever engine is less
                        # busy (greedy min-makespan)
                        eligible = (not has_diag) and (q0 + c0) >= 512
                        cost_s = w * _NS_COL_S + _OV_S
                        cost_d = w * _NS_COL_D + _OV_D
                        if eligible and busy["D"] + cost_d <= busy["S"] + cost_s:
                            busy["D"] += cost_d
                            nc.vector._custom_dve(
                                EXP16,
                                out=dst,
                                in0=sc[:, lo_h:lo_h + w],
                                s0=float(C0d if fp8_chunk else B2),
                                s1=float(C1d if fp8_chunk else B1),
                                imm2=float(C2d if fp8_chunk else B0),
                            )
                        else:
                            busy["S"] += cost_s
                            nc.scalar.activation(
                                dst,
                                sc[:, lo_h:lo_h + w],
                                mybir.ActivationFunctionType.Exp,
                                scale=16.0,
                                bias=nb_t[:] if fp8_chunk else 0.0,
                            )
                        if i + LOOKAHEAD < len(halves):
                            emit_half_scores(i + LOOKAHEAD)
                        # PV matmuls for the sub-q blocks of this half
                        if fp8_chunk and kb % 2 == 0:
                            continue  # fp8 PVs fire on the pair's odd strip
                        if fp8_chunk:
                            t = kb // 2
                            pv_lo = max((kb - 1) * KB - q0, 0)
                        else:
                            pv_lo = c0
                        for s in range(hh * 4, hh * 4 + 4):
                            qs0 = s * 128
                            if qs0 < pv_lo:
                                continue  # sub-q fully masked for this k block
                            if fp8_chunk:
                                stop_t = (q0 // KB + s) // 2
                                nc.tensor.matmul(
                                    ctx_ap(s),
                                    pt_tiles[t][:, :, qs0:qs0 + 128],
                                    va2_t[:, t, :, :],
                                    start=(t == 0 and s % 3 == 0),
                                    stop=(t == stop_t),
                                    perf_mode=mybir.MatmulPerfMode.DoubleRow,
                                    skip_group_check=True,
                                )
                                is_stop = t == stop_t
                            else:
                                last_kb = q0 // KB + s
                                nc.tensor.matmul(
                                    ctx_ap(s),
                                    pt_tiles[kb][:, qs0:qs0 + 128],
                                    va_t[:, kb, :],
                                    start=(kb == 0 and s % 3 == 0),
                                    stop=(kb == last_kb),
                                    skip_group_check=True,
                                )
                                is_stop = kb == last_kb
                            # normalize + store a ctx bank as soon as its
                            # last accumulation group stopped
                            for bank, s_hi in ((0, 2), (1, 5), (2, 7)):
                                if s != s_hi or not is_stop:
                                    continue
                                s_lo = 3 * bank
                                nsb = s_hi - s_lo + 1
                                # drain the bank PSUM->SBUF in one batched
                                # copy (ScalarE or DVE, whichever is lighter),
                                # reciprocal from SBUF on DVE, then the idle
                                # GpSimd applies 1/l with a free-dim-broadcast
                                # tensor multiply (GpSimd cannot read PSUM,
                                # and its tensor_scalar path is slow, but
                                # plain tensor_tensor runs at ~0.42 eff)
                                raw = o_pool.tile(
                                    [128, 3, KB + 1], F32, tag="raw", name="raw"
                                )
                                ob = o_pool.tile([128, 3, D], BF16, tag="ob")
                                rec = r_pool.tile([128, 3], F32, tag="rec")
                                cb = ctx_tiles[bank][
                                    :, 0:nsb * (KB + 1)
                                ].rearrange("p (g c) -> p g c", c=KB + 1)
                                cost_s = nsb * (KB + 1) * _NS_COL_S + _OV_S
                                cost_d = nsb * (KB + 1) * _NS_COL_D + _OV_D
                                if busy["S"] + cost_s <= busy["D"] + cost_d:
                                    busy["S"] += cost_s
                                    nc.scalar.copy(raw[:, 0:nsb, :], cb)
                                else:
                                    busy["D"] += cost_d
                                    nc.vector.tensor_copy(raw[:, 0:nsb, :], cb)
                                busy["D"] += _OV_D
                                nc.vector.reciprocal(
                                    rec[:, 0:nsb], raw[:, 0:nsb, KB]
                                )
                                for j in range(nsb):
                                    nc.gpsimd.tensor_mul(
                                        ob[:, j, :],
                                        raw[:, j, 0:D],
                                        rec[:, j:j + 1].to_broadcast([128, D]),
                                    )
                                g_lo = q0 // KB + s_lo
                                nc.sync.dma_start(
                                    out=out_d[p, :, g_lo:g_lo + nsb, :],
                                    in_=ob[:, 0:nsb, :],
                                )
    nc.compile()
    return nc


def _prep_inputs(query_layer, key_layer, value_layer):
    q = np.asarray(query_layer, dtype=np.float32).reshape(NPAIRS, S, D)
    k = np.asarray(key_layer, dtype=np.float32).reshape(NPAIRS, S, D)
    v = np.asarray(value_layer, dtype=np.float32).reshape(NPAIRS, S, D)

    qt = np.ascontiguousarray((q * np.float32(PRE)).transpose(0, 2, 1)).astype(
        ml_dtypes.bfloat16
    )
    kt = np.ascontiguousarray(k.transpose(0, 2, 1)).astype(ml_dtypes.bfloat16)
    vk = v.reshape(NPAIRS, NKT, KB, D).transpose(0, 2, 1, 3)  # [N, k, kb, D]
    # chunk-0 (rows < 1024) PV operand: bf16 [V | 1], k blocks 0..7
    va = np.ones((NPAIRS, KB, NKT // 2, KB + 1), dtype=ml_dtypes.bfloat16)
    va[:, :, :, :D] = vk[:, :, : NKT // 2].astype(ml_dtypes.bfloat16)
    # chunk-1 (rows >= 1024) PV operand: fp8 [V | 1], all k blocks packed in
    # pairs for DoubleRow (contraction 2x128)
    fp8 = ml_dtypes.float8_e4m3fn
    va2 = np.ones((NPAIRS, KB, NKT // 2, 2, KB + 1), dtype=fp8)
    va2[:, :, :, :, :D] = (
        vk.reshape(NPAIRS, KB, NKT // 2, 2, D).astype(fp8)
    )
    in_maps = [
        {
            "qt": np.ascontiguousarray(qt[c * PPC:(c + 1) * PPC]),
            "kt": np.ascontiguousarray(kt[c * PPC:(c + 1) * PPC]),
            "va": np.ascontiguousarray(va[c * PPC:(c + 1) * PPC]),
            "va2": np.ascontiguousarray(va2[c * PPC:(c + 1) * PPC]),
        }
        for c in range(NCORES)
    ]
    return in_maps


def _run(query_layer, key_layer, value_layer, trace=False):
    in_maps = _prep_inputs(query_layer, key_layer, value_layer)
    nc = _build_nc()
    res = run_bass_kernel_spmd(nc, in_maps, list(range(NCORES)), trace=trace)
    # device layout [core, pair, q%128, q//128, d] -> [b, s, h*d]
    ctx = np.stack([res.results[c]["out"] for c in range(NCORES)])
    ctx = ctx.transpose(0, 1, 3, 2, 4)  # [core, pair, q//128, q%128, d]
    out = ctx.reshape(B, H, S, D).transpose(0, 2, 1, 3).reshape(B, S, H * D)
    return np.ascontiguousarray(out, dtype=np.float32), res


def kernel(query_layer, key_layer, value_layer):
    out, _ = _run(query_layer, key_layer, value_layer, trace=False)
    return out
